# revision 16
# baseline (speedup 1.0000x reference)
"""Ex2Vec Trainium2 Bass kernel (v2: batched dma_gather + transposed tables).

Data-parallel over batch B=32 across 8 NeuronCores (4 batch rows/core).

Math (per batch row b):
    u   = emb_user[user]                  [D]
    p   = emb_item[pred]                  [P, D]
    h   = emb_item[hist]                  [H, D]
    sq[i,j]   = |h_i|^2 + |p_j|^2 - 2 h_i.p_j   (+EPS2 folded in)
    dist      = sqrt(sq)
    kern      = sigmoid(smooth/(1+dist) - force*smooth) / denom
    td        = (t + cutoff)^-.5 * w * (global_lamb + user_lamb[u]) / denom
    res_j     = sum_i td_i kern[i,j]
    dist_ui_j = sqrt(|u-p_j|^2 + EPS2)
    out       = relu(dist_ui - res)
    I         = alpha*out + beta*out^2 + gamma + user_bias[u] + item_bias[pred]

Device strategy:
  - Host preps two bf16 tables of 256B rows with augmentation baked in:
      tab_p[v] = [emb | 1 | |emb|^2 | bias | 0...]          (rhs form)
      tab_h[v] = [-2*emb | |emb|^2+EPS2 | 1 | 0...]         (lhsT form)
    and a f32 user table tab_u[v] = [-2*emb | uu+EPS2 | 1 | lamb* | gamma+ub].
  - Gathers use InstDMAGatherAnt (one instruction per bank, ~1us fixed cost,
    vs ~1.1us per 128 rows with InstDMACopy): pred indices are bank-compacted
    host-side into 4 banks of 32768 rows (int16 index limit); hist rows ride
    a 4-row-block gather (idx>>2 <= 25000 fits int16).
  - A second SBUF-source dma_gather (transpose=True) un-permutes the banked
    rows AND transposes them so embedding dims land on partitions: the Gram
    matmul needs NO tensor-engine transposes at all, and the augmented rows
    make PSUM hold finished squared distances.
  - kern chain is 3 passes: ACT Sqrt (psum->bf16), DVE (1+d)->recip, ACT
    Sigmoid. res/du/bias reductions are masked-lhsT matmuls as in v1.
"""

import os
import numpy as np
import ml_dtypes
from contextlib import ExitStack

import concourse.bass as bass
import concourse.bacc as bacc
import concourse.mybir as mybir
import concourse.tile as tile
from concourse.masks import make_identity
from concourse.bass_utils import run_bass_kernel_spmd

F32 = mybir.dt.float32
BF16 = mybir.dt.bfloat16
I16 = mybir.dt.int16
I32 = mybir.dt.int32
AF = mybir.ActivationFunctionType
OP = mybir.AluOpType
AX = mybir.AxisListType
BF = ml_dtypes.bfloat16

NCORES = 8
B = 32
BPC = B // NCORES          # 4 batch rows per core
P_REAL = 1000
PP = 1024                  # padded pred count per batch row
NSLOT = BPC * PP           # 4096 pred slots per core
H = 128
D = 64
V = 100001
VH = 100004                # tab_h padded to a multiple of 4 rows
EPS2 = 0.02                # distance epsilon (absorbs bf16 matmul rounding)
BANK = 32768
NBANK = 4
EW = 128                   # bf16 elems per table row = 256B
HBLK = 512                 # hist rows per core (BPC*H)

_cache: dict = {}


def _wrap16(a, width):
    """int array -> [128, width] int16 (idx j at [j%16, j//16], replicated
    down the other 112 partitions)."""
    m = np.zeros((16, width), np.int16)
    n = len(a)
    if n:
        m[np.arange(n) % 16, np.arange(n) // 16] = np.asarray(a, np.int16)
    return np.tile(m, (8, 1))


def _build(scalars, banks):
    """banks = (B0..B3): per-bank block counts (compile-time constants)."""
    (global_lamb, alpha, beta, gamma, cutoff, smooth, force) = scalars
    inv_smooth = float(1.0 / smooth)
    neg_fs = float(-force * smooth)
    NPB = sum(banks)                    # pred region blocks (= 256B ranks)
    R = NPB + 16                        # + hist region: 512 blocks*1KB = 16 ranks
    PBASE = [128 * sum(banks[:k]) for k in range(NBANK)]   # bank token starts
    NCHK = 2304                         # G2 chunk: 2 batch rows of p + their h

    nc = bacc.Bacc("TRN2", target_bir_lowering=False, debug=False,
                   num_devices=NCORES)

    for v in sorted({float(cutoff), float(neg_fs)}):
        if (F32, v) not in nc.const_aps.aps:
            t = nc.alloc_sbuf_tensor(f"constap-{v}", [128, 1], F32)
            nc.gpsimd.memset(t.ap(), v)
            nc.const_aps.aps[(F32, v)] = t.ap()
    nc.all_engine_barrier()

    # ---- DRAM I/O ------------------------------------------------------
    t_tabp = nc.dram_tensor("tab_p", [V, EW], BF16, kind="ExternalInput")
    t_tabh = nc.dram_tensor("tab_h", [VH // 4, 4 * EW], BF16,
                            kind="ExternalInput")
    t_tabu = nc.dram_tensor("tab_u", [V, 69], F32, kind="ExternalInput")
    t_idxp = [nc.dram_tensor(f"idxp{k}", [128, banks[k] * 8], I16,
                             kind="ExternalInput") for k in range(NBANK)]
    t_idxh = nc.dram_tensor("idxh", [128, HBLK // 16], I16,
                            kind="ExternalInput")
    t_idx2 = [nc.dram_tensor(f"idx2{cjk}", [128, NCHK // 16], I16,
                             kind="ExternalInput") for cjk in range(2)]
    t_idxu = nc.dram_tensor("idx_user", [BPC, 1], I32, kind="ExternalInput")
    t_td = nc.dram_tensor("tdelta", [BPC, H], F32, kind="ExternalInput")
    t_wt = nc.dram_tensor("tweight", [BPC, H], F32, kind="ExternalInput")
    t_out = nc.dram_tensor("out", [BPC, P_REAL], F32, kind="ExternalOutput")

    with tile.TileContext(nc) as tc, ExitStack() as ctx:
        const = ctx.enter_context(tc.tile_pool(name="const", bufs=1))
        sb = ctx.enter_context(tc.tile_pool(name="sb", bufs=1))
        dpool = ctx.enter_context(tc.tile_pool(name="dpool", bufs=4))
        kpool = ctx.enter_context(tc.tile_pool(name="kpool", bufs=4))
        ps_g = ctx.enter_context(tc.tile_pool(name="ps_g", bufs=4, space="PSUM"))
        ps_d = ctx.enter_context(tc.tile_pool(name="ps_d", bufs=2, space="PSUM"))
        ps_r = ctx.enter_context(tc.tile_pool(name="ps_r", bufs=2, space="PSUM"))

        ident = const.tile([128, 128], F32)
        make_identity(nc, ident[:])

        # ---- small input loads (HWDGE on sync engine) ------------------
        idxp_sb = [sb.tile([128, banks[k] * 8], I16, name=f"idxp{k}")
                   for k in range(NBANK)]
        idxh_sb = sb.tile([128, HBLK // 16], I16)
        idx2_sb = [sb.tile([128, NCHK // 16], I16, name=f"idx2{c}")
                   for c in range(2)]
        idxu_sb = sb.tile([BPC, 1], I32)
        td_sb = sb.tile([BPC, H], F32)
        wt_sb = sb.tile([BPC, H], F32)
        for k in range(NBANK):
            nc.sync.dma_start(out=idxp_sb[k][:], in_=t_idxp[k][:])
        nc.sync.dma_start(out=idxh_sb[:], in_=t_idxh[:])
        for c in range(2):
            nc.sync.dma_start(out=idx2_sb[c][:], in_=t_idx2[c][:])
        nc.sync.dma_start(out=idxu_sb[:], in_=t_idxu[:])
        nc.sync.dma_start(out=td_sb[:], in_=t_td[:])
        nc.sync.dma_start(out=wt_sb[:], in_=t_wt[:])

        # ---- gathers (all gpsimd) --------------------------------------
        gbuf = sb.tile([128, R * EW], BF16)
        gb3 = gbuf[:].rearrange("p (a b) -> p a b", b=EW)
        for k in range(NBANK):
            nrows = min(BANK, V - k * BANK)
            nc.gpsimd.dma_gather(
                out_ap=gb3[:, PBASE[k] // 128:PBASE[k] // 128 + banks[k], :],
                in_ap=t_tabp[k * BANK:k * BANK + nrows, :],
                idxs_ap=idxp_sb[k][:],
                num_idxs=banks[k] * 128, num_idxs_reg=banks[k] * 128,
                elem_size=EW, single_packet=False)
        # hist: 4-row blocks (1KB each), block idx = hidx>>2
        hview = gbuf[:, NPB * EW:(NPB + 16) * EW].rearrange(
            "p (a b) -> p a b", b=4 * EW)
        nc.gpsimd.dma_gather(
            out_ap=hview, in_ap=t_tabh[:],
            idxs_ap=idxh_sb[:],
            num_idxs=HBLK, num_idxs_reg=HBLK, elem_size=4 * EW, single_packet=False)
        # user rows (f32, classic indirect gather)
        uau = sb.tile([BPC, 69], F32)
        nc.gpsimd.indirect_dma_start(
            out=uau[:], out_offset=None, in_=t_tabu[:],
            in_offset=bass.IndirectOffsetOnAxis(ap=idxu_sb[:, :1], axis=0))

        # ---- user/td prep (overlaps gathers) ---------------------------
        ps_u = ps_d.tile([69, BPC], F32, space="PSUM", tag="du")
        nc.tensor.transpose(ps_u[0:69, 0:BPC], uau[:], ident[0:BPC, 0:BPC])
        # masked lhsT columns: [-2u | 0 | 1 | uu+EPS2] per batch row
        uT4 = sb.tile([67, BPC * BPC], BF16)
        nc.vector.memset(uT4[:].bitcast(F32), 0.0)
        for b in range(BPC):
            col = slice(BPC * b + b, BPC * b + b + 1)
            nc.vector.tensor_copy(uT4[0:67, col], ps_u[0:67, b:b + 1])
        ubg = uau[:, 68:69]                           # gamma + user_bias [4,1]
        lamb_c = uau[:, 67:68]                        # (gl+ulamb)/denom [4,1]

        tds = sb.tile([BPC, H], F32)
        tdr = sb.tile([BPC, H], F32)
        tdl = sb.tile([BPC, H], F32)
        nc.scalar.activation(tds[:], td_sb[:], AF.Sqrt, bias=float(cutoff))
        nc.vector.reciprocal(tdr[:], tds[:])
        nc.vector.scalar_tensor_tensor(
            out=tdl[:], in0=tdr[:], scalar=lamb_c[:, :1], in1=wt_sb[:],
            op0=OP.mult, op1=OP.mult)
        ps_t = ps_r.tile([128, BPC], F32, space="PSUM", tag="racc")
        nc.tensor.transpose(ps_t[:, 0:BPC], tdl[:], ident[0:BPC, 0:BPC])
        td4m = sb.tile([128, BPC * BPC], BF16)
        nc.vector.memset(td4m[:].bitcast(F32), 0.0)
        for b in range(BPC):
            nc.vector.tensor_copy(td4m[:, BPC * b + b:BPC * b + b + 1],
                                  ps_t[:, b:b + 1])
        # masked ones lhsT at base partition 64 (pairs the bias row of tab_p)
        ones4 = sb.tile([65, BPC * BPC], BF16)
        nc.vector.memset(ones4[64:65, :].bitcast(F32), 0.0)
        for b in range(BPC):
            nc.gpsimd.memset(ones4[64:65, BPC * b + b:BPC * b + b + 1], 1.0)

        # ---- G2: transposed un-permute (2 chunks of 2 batch rows) ------
        tg = [sb.tile([128, 1, NCHK], BF16, name=f"tg{c}") for c in range(2)]
        for c in range(2):
            nc.gpsimd.dma_gather(
                out_ap=tg[c][:], in_ap=gbuf[:],
                idxs_ap=idx2_sb[c][:],
                num_idxs=NCHK, num_idxs_reg=NCHK, elem_size=EW,
                single_packet=False, transpose=True,
                sbuf_tokens_per_rank=128,
                sbuf_free_dim_per_rank=EW * 2)

        # per (b,g): p cols + h col inside the right tg chunk
        def pcols(b, g):
            return tg[b // 2][0:67, 0, (b % 2) * PP + g * 512:
                              (b % 2) * PP + (g + 1) * 512]

        def prow64(b, g):
            return tg[b // 2][64:65, 0, (b % 2) * PP + g * 512:
                              (b % 2) * PP + (g + 1) * 512]

        def hcols(b):
            return tg[b // 2][0:67, 0, 2 * PP + (b % 2) * H:
                              2 * PP + (b % 2 + 1) * H]

        # ---- du^2 and bias broadcasts (masked-lhsT matmuls) ------------
        psD = [ps_d.tile([BPC, 512], F32, space="PSUM", tag="du",
                         name=f"psD{g}") for g in range(2)]
        psB = [ps_g.tile([BPC, 512], F32, space="PSUM", tag="gram",
                         name=f"psB{g}") for g in range(2)]
        for b in range(BPC):
            for g in range(2):
                nc.tensor.matmul(psD[g][:],
                                 lhsT=uT4[:, BPC * b:BPC * (b + 1)],
                                 rhs=pcols(b, g),
                                 start=(b == 0), stop=(b == BPC - 1),
                                 skip_group_check=True)
                nc.tensor.matmul(psB[g][:],
                                 lhsT=ones4[64:65, BPC * b:BPC * (b + 1)],
                                 rhs=prow64(b, g),
                                 start=(b == 0), stop=(b == BPC - 1),
                                 skip_group_check=True)
        duS = sb.tile([BPC, PP], F32)
        biasS = sb.tile([BPC, PP], F32)
        for g in range(2):
            nc.scalar.activation(duS[:, g * 512:(g + 1) * 512], psD[g][:],
                                 AF.Sqrt)
            nc.vector.tensor_copy(biasS[:, g * 512:(g + 1) * 512], psB[g][:])

        # ---- Gram + kern chain -----------------------------------------
        dt_tiles = {}
        for b in range(BPC):
            for g in range(2):
                mm = ps_g.tile([128, 512], F32, space="PSUM", tag="gram")
                nc.tensor.matmul(mm[:], lhsT=hcols(b), rhs=pcols(b, g),
                                 start=True, stop=True)
                dt_ = dpool.tile([128, 512], BF16, tag="dt")
                nc.scalar.activation(dt_[:], mm[:], AF.Sqrt)
                dt_tiles[(b, g)] = dt_
        r_tiles = {}
        with nc.allow_low_precision(reason="res tolerates bf16 (relu clamp)"):
            for b in range(BPC):
                for g in range(2):
                    t1 = kpool.tile([128, 512], BF16, tag="t1")
                    nc.vector.tensor_scalar(t1[:], dt_tiles[(b, g)][:],
                                            inv_smooth, inv_smooth,
                                            op0=OP.mult, op1=OP.add)
                    rt = kpool.tile([128, 512], BF16, tag="rt")
                    nc.vector.reciprocal(rt[:], t1[:])
                    r_tiles[(b, g)] = rt
        psR = [ps_r.tile([BPC, 512], F32, space="PSUM", tag="racc",
                         name=f"psR{g}") for g in range(2)]
        for b in range(BPC):
            for g in range(2):
                kern_t = dpool.tile([128, 512], BF16, tag="kern")
                nc.scalar.activation(kern_t[:], r_tiles[(b, g)][:],
                                     AF.Sigmoid, bias=neg_fs)
                nc.tensor.matmul(psR[g][:],
                                 lhsT=td4m[:, BPC * b:BPC * (b + 1)],
                                 rhs=kern_t[:],
                                 start=(b == 0), stop=(b == BPC - 1),
                                 skip_group_check=True)

        # ---- finals: I = (alpha + beta*o)*o + (gamma+ub) + ib ----------
        irows = sb.tile([BPC, PP], F32)
        for g in range(2):
            sl = slice(g * 512, (g + 1) * 512)
            o1 = sb.tile([BPC, 512], F32, tag="o1")
            o = sb.tile([BPC, 512], F32, tag="o")
            q = sb.tile([BPC, 512], F32, tag="q")
            m = sb.tile([BPC, 512], F32, tag="m")
            nc.vector.tensor_sub(o1[:], duS[:, sl], psR[g][:])
            nc.gpsimd.tensor_scalar_max(o[:], o1[:], 0.0)
            nc.vector.tensor_scalar(q[:], o[:], float(beta), float(alpha),
                                    op0=OP.mult, op1=OP.add)
            nc.gpsimd.tensor_mul(m[:], q[:], o[:])
            nc.vector.scalar_tensor_tensor(
                out=irows[:, sl], in0=m[:], scalar=ubg[:, :1],
                in1=biasS[:, sl], op0=OP.add, op1=OP.add)
        nc.sync.dma_start(out=t_out[:], in_=irows[:, 0:P_REAL])

    nc.compile()
    return nc


def _prep(inputs):
    """Host-side tables + per-core index/bank prep."""
    user_index = np.asarray(inputs["user_index"]).astype(np.int64)
    pred = np.asarray(inputs["pred_item_indices"]).astype(np.int64)
    hist = np.asarray(inputs["history_item_indices"]).astype(np.int64)
    tdelta = np.asarray(inputs["history_timedeltas"], dtype=np.float32)
    weights = np.asarray(inputs["history_weights"], dtype=np.float32)
    emb_user = np.asarray(inputs["embedding_user"], dtype=np.float32)
    emb_item = np.asarray(inputs["embedding_item"], dtype=np.float32)
    user_lamb = np.asarray(inputs["user_lamb"], dtype=np.float32)
    user_bias = np.asarray(inputs["user_bias"], dtype=np.float32)
    item_bias = np.asarray(inputs["item_bias"], dtype=np.float32)
    gl, alpha, beta, gamma, cutoff, smooth, force = (
        float(np.asarray(inputs[k])) for k in
        ("global_lamb", "alpha", "beta", "gamma", "cutoff", "smooth", "force"))
    denom = 1.0 / (1.0 + np.exp(-(smooth - force * smooth)))

    # layouts (as gathered-transposed partitions):
    #   tab_p: [emb | bias(64) | normsq(65) | 1(66)]
    #   tab_h: [-2emb | 0(64) | 1(65) | hh+EPS2(66)]
    #   tab_u: [-2u | 0(64) | 1(65) | uu+EPS2(66) | lamb*(67) | gamma+ub(68)]
    nsq = np.sum(emb_item * emb_item, axis=1, keepdims=True)
    tab_p = np.zeros((V, EW), BF)
    tab_p[:, 0:64] = emb_item
    tab_p[:, 64:65] = item_bias
    tab_p[:, 65:66] = nsq
    tab_p[:, 66] = 1.0
    tab_h = np.zeros((VH, EW), BF)
    tab_h[:V, 0:64] = -2.0 * emb_item
    tab_h[:V, 65] = 1.0
    tab_h[:V, 66:67] = nsq + EPS2
    tab_u = np.zeros((V, 69), np.float32)
    tab_u[:, 0:64] = -2.0 * emb_user
    tab_u[:, 65] = 1.0
    tab_u[:, 66:67] = np.sum(emb_user * emb_user, 1, keepdims=True) + EPS2
    tab_u[:, 67:68] = (gl + user_lamb) / denom
    tab_u[:, 68:69] = gamma + user_bias

    pred_pad = np.zeros((B, PP), np.int64)
    pred_pad[:, :P_REAL] = pred

    cores = []
    for c in range(NCORES):
        sl = slice(c * BPC, (c + 1) * BPC)
        pidx = pred_pad[sl].reshape(-1)               # [4096] slot-ordered
        bank = (pidx >> 15).astype(np.int64)
        lists, poss = [], np.zeros(NSLOT, np.int64)
        for k in range(NBANK):
            mask = bank == k
            lists.append((pidx[mask] - k * BANK).astype(np.int64))
            poss[mask] = np.arange(mask.sum())
        hidx = hist[sl].reshape(-1)                   # [512]
        cores.append(dict(pidx=pidx, bank=bank, lists=lists, poss=poss,
                          hidx=hidx, sl=sl))

    # common per-bank block counts across cores (program is SPMD-shared)
    banks = tuple(max(1, max(-(-len(cr["lists"][k]) // 128) for cr in cores))
                  for k in range(NBANK))
    NPB = sum(banks)
    PBASE = np.asarray([128 * sum(banks[:k]) for k in range(NBANK)],
                       np.int64)

    in_maps = []
    for cr in cores:
        # pad bank lists with dummy index 0 to the common block counts
        idxp = []
        for k in range(NBANK):
            lst = np.zeros(banks[k] * 128, np.int64)
            lst[:len(cr["lists"][k])] = cr["lists"][k]
            idxp.append(_wrap16(lst, banks[k] * 8))
        # hist block gather: block idx, sub-row
        hblk = cr["hidx"] >> 2
        hsub = cr["hidx"] & 3
        idxh = _wrap16(hblk, HBLK // 16)
        # G2 tokens: pred slot -> bank region + position; hist slot -> block
        ptok = PBASE[cr["bank"]] + cr["poss"]
        s = np.arange(HBLK)
        htok = (NPB + 4 * (s >> 7) + hsub) * 128 + (s & 127)
        idx2 = []
        for half in range(2):
            t = np.concatenate([ptok[half * 2 * PP:(half + 1) * 2 * PP],
                                htok[half * 2 * H:(half + 1) * 2 * H]])
            idx2.append(_wrap16(t, 2304 // 16))
        im = {"tab_p": tab_p, "tab_h": tab_h.reshape(VH // 4, 4 * EW),
              "tab_u": tab_u,
              "idxh": idxh, "idx_user":
              np.ascontiguousarray(user_index[cr["sl"], None].astype(np.int32)),
              "tdelta": tdelta[cr["sl"]], "tweight": weights[cr["sl"]]}
        for k in range(NBANK):
            im[f"idxp{k}"] = idxp[k]
        for half in range(2):
            im[f"idx2{half}"] = idx2[half]
        in_maps.append(im)
    return (gl, alpha, beta, gamma, cutoff, smooth, force), banks, in_maps


def kernel(**inputs) -> np.ndarray:
    scalars, banks, in_maps = _prep(inputs)
    key = (tuple(float(s) for s in scalars), banks)
    if key not in _cache:
        _cache[key] = _build(key[0], banks)
    nc = _cache[key]

    res = run_bass_kernel_spmd(
        nc, in_maps, core_ids=list(range(NCORES)),
        trace=bool(int(os.environ.get("K_TRACE", "0"))))
    if res.exec_time_ns is not None:
        kernel.last_exec_time_ns = res.exec_time_ns
    kernel.last_results = res

    out = np.concatenate([res.results[c]["out"] for c in range(NCORES)],
                         axis=0)
    return np.ascontiguousarray(out, dtype=np.float32)


if __name__ == "__main__":
    import reference
    inputs = {k: np.asarray(v) for k, v in reference.setup_inputs().items()}
    expected = np.asarray(reference.reference(**reference.setup_inputs()))
    actual = kernel(**inputs)
    err = np.abs(actual - expected)
    rel = err.max() / np.abs(expected).max()
    print("max abs err:", err.max(), "rel:", rel)
